# revision 14
# baseline (speedup 1.0000x reference)
"""AttentionWithRoPE distributed Trainium2 kernel (8 NeuronCores).

Sharding: pure 8-way tensor parallel over heads (2 heads = 128 hidden cols
per core), both batches on every core (seq concatenated to 4096 cols).
Everything stays transposed ([feature, seq] layouts) so no on-device
transposes are needed anywhere.

The kernel is ScalarE-bound: exp of the 2x[2048,2048] score matrices is
~142us of ACTIVATE at 1 elem/lane/cycle. The schedule keeps ScalarE
saturated from ~30us on:
  phase A: project q/k/v + rope for batch 0 (seq groups 0-3).
  phase C: 4 attention passes for batch 0. ALL batch-1 projection work
           (q/k matmuls, rope, v) is spread as small insertions BETWEEN
           KEY-STEPS INSIDE the passes (between passes ScalarE has no exp
           backlog, so any block there stalls it 1:1).
  phase D: 4 attention passes for batch 1, with keep-warm dummy matmuls
           in every key-step: without them the ACT-gated PE micro-idles
           every ~700ns, HAM re-throttles the PE clock to 1.2GHz, and the
           slowed matmuls then gate ACT (observed k=4/8 / k=13/16
           oscillation, never 8/8).
  phase E: AllToAll + keep-warm matmul chain + output projection.

Every dma_start costs ~600ns of SWDGE descriptor time ON THE ISSUING
ENGINE, so DMAs are few and fat, spread over the Sync/GpSimd/ScalarE
queues: x streams as 16x [128,2048] chunks over all three, wq|wk|wv are
host-packed into one [128,3072] load, Wo into one [128,8192], cos|sin in
4 pieces, biases in one. The rope half-rotation (a 32-row partition swap,
which DVE cannot do: ops must keep operand start-partitions equal) is a
PE matmul against a host-supplied 0/1 permutation matrix; the sin-multiply
reads the swapped copy straight from PSUM.

Attention details:
  - scores^T = kT.T @ qT per (head, batch) in [ks, qs] layout as K=64
    row-tiled matmul pairs: head0 streams through PE rows 0-63 while head1
    streams through rows 64-127 concurrently (auto tile_position (0,0) /
    (64,0) from the operands' base partitions; verified dStart ~3ns).
  - exp on ScalarE in [128,1024] ops over 2-bank psum score tiles (~1113ns
    each; 1536-wide ops measure WORSE - 3-bank PSUM reads pay ~230ns).
  - ctx^T via M=65 matmuls with a ones-column appended to V (the 65th
    column gives the softmax denominator for free). V lives in a single 3D
    tile [128, 64, 65] (slot = key-block*2+head); ones columns initialized
    by ONE strided memset.
  - normalization: rowsum (psum partition 64) -> sbuf, DMA-reshape to
    [128,4] so reciprocal runs 128 lanes wide (single-partition reciprocal
    is ~8 cyc/elem!), DMA back, GpSimd partition-broadcast, one DVE
    multiply (fuses psum->sbuf copy + cast). Hop DMAs ride the idle Sync
    queue.
  - PSUM: 2x 2-bank score slots + 3x 1-bank accumulator slots (q/k proj,
    rope swap, v psums, the two ctx accumulators) + 1 dummy bank = 8.
  - AllToAll (bf16, all 8 cores) exchanges 512-row blocks of ctx^T;
    received slabs are exactly the o-chunks the output projection consumes.
  - output projection with full Wo produces out^T [1024, 512] for this
    core's 512 global rows; host transposes back (free).
Bias folds (host side): v-bias folds into the output bias exactly (softmax
rows sum to 1); q is pre-scaled by 1/sqrt(64) inside its bias-copy.
Compute dtype bf16 (fp32 PSUM accumulation).
"""

import numpy as np

HID = 1024
S = 2048
SB = 2 * S       # both batches, seq-concatenated
NHEAD = 16
D = 64
HPC = 2          # heads per core
OSL = 128        # hidden slice per core (HPC * D)
RB = 512         # global row block per core after AllToAll
NC = 8
ROPE_BASE = 10000.0

_cached = None
_last_in_maps = None

N_DUMMY = 150    # keep-PE-warm matmuls spanning the AllToAll wait


def _build_nc():
    import concourse.bacc as bacc
    import concourse.mybir as mybir
    from concourse import tile

    f32 = mybir.dt.float32
    bf16 = mybir.dt.bfloat16
    AF = mybir.ActivationFunctionType

    nc = bacc.Bacc(None, target_bir_lowering=False)

    xT = nc.declare_dram_parameter("xT", [HID, SB], bf16, isOutput=False)
    wqkvd = nc.declare_dram_parameter("wqkv", [128, 3 * HID], bf16,
                                      isOutput=False)
    wod = nc.declare_dram_parameter("woL", [128, 8 * HID], bf16,
                                    isOutput=False)
    bqkd = nc.declare_dram_parameter("bqk", [128, 2], f32, isOutput=False)
    bod = nc.declare_dram_parameter("bo2", [128, 8], f32, isOutput=False)
    csd = nc.declare_dram_parameter("cs", [128, 2 * SB], bf16,
                                    isOutput=False)
    permd = nc.declare_dram_parameter("perm", [128, 128], bf16,
                                      isOutput=False)
    out_ext = nc.declare_dram_parameter("out", [HID, RB], bf16, isOutput=True)

    a2a_in = nc.dram_tensor("a2a_in", [NC, OSL, RB], bf16)
    a2a_out = nc.dram_tensor("a2a_out", [NC, OSL, RB], bf16)

    NHC = HID // 128  # 8 hidden chunks
    QENG = None       # set inside

    with tile.TileContext(nc) as tc:
        with (
            tc.tile_pool(name="persist", bufs=1) as pp,
            tc.tile_pool(name="xs", bufs=16) as xp,
            tc.tile_pool(name="work", bufs=2) as wp,
            tc.tile_pool(name="exp", bufs=2) as ep,
        ):
            # ---------- consts: fat DMAs spread over the 3 queues ---------
            wqkv = pp.tile([128, 3 * HID], bf16, tag="wqkv", name="wqkv")
            nc.scalar.dma_start(out=wqkv[:, :], in_=wqkvd[:, :])
            bqk = pp.tile([128, 2], f32, tag="bqk", name="bqk")
            nc.scalar.dma_start(out=bqk[:, :], in_=bqkd[:, :])
            perm = pp.tile([128, 128], bf16, tag="perm", name="perm")
            nc.scalar.dma_start(out=perm[:, :], in_=permd[:, :])
            cs = pp.tile([128, 2 * SB], bf16, tag="cs", name="cs")
            for half in range(2):          # cos-b0, sin-b0, cos-b1, sin-b1
                for part in range(2):
                    lo = SB * part + S * half
                    nc.scalar.dma_start(out=cs[:, lo:lo + S],
                                        in_=csd[:, lo:lo + S])
            bo_sb = pp.tile([128, 8], f32, tag="bo", name="bo")
            nc.scalar.dma_start(out=bo_sb[:, :], in_=bod[:, :])

            def wsl(t, c):      # lhsT slice for projection t in (q,k,v)
                return wqkv[:, 1024 * t + 128 * c:1024 * t + 128 * (c + 1)]

            # x: 16 fat [128,2048] chunks over 3 queues; pair 0 = batch 0
            # columns, pair 1 = batch 1.
            xq = {}
            for p in range(2):
                tiles = []
                for c in range(NHC):
                    xb = xp.tile([128, 2048], bf16, tag="xb", bufs=16)
                    (nc.sync if p == 0 else nc.gpsimd).dma_start(
                        out=xb[:, :],
                        in_=xT[128 * c:128 * (c + 1),
                               2048 * p:2048 * (p + 1)])
                    tiles.append(xb)
                xq[p] = tiles

            # PSUM pools (8 banks exactly):
            #  psA "spsbig": 2x [128,1024] (scores)          -> 4 banks
            #  psB "acc":    3x [128,512]  (proj/swap/ctx)   -> 3 banks
            #  psD "dumA":   1x [128,512]  (keep-warm)       -> 1 bank
            _cmA = tc.tile_pool(name="psA", bufs=3, space="PSUM")
            _cmB = tc.tile_pool(name="psB", bufs=2, space="PSUM")
            psA = _cmA.__enter__()
            psB = _cmB.__enter__()

            qr = pp.tile([128, SB], bf16, tag="qr", name="qr")
            kr = pp.tile([128, SB], bf16, tag="kr", name="kr")
            vt = pp.tile([128, 2 * SB // 128, D + 1], bf16, tag="vt",
                         name="vt")
            nc.gpsimd.memset(vt[:, :, D:D + 1], 1.0)
            ctxh = [pp.tile([64, SB], bf16, tag=f"ctx{h}", name=f"ctx{h}")
                    for h in range(HPC)]

            # ---------- emission helpers (all emit small blocks) ----------
            # qh/kh: post-bias pre-rope 512-col blocks, consumed by rope.
            def qk_sg2(sg, xbt, xlo):
                ps = psA.tile([128, 1024], f32, tag="spsbig",
                              name=f"qkps{sg}")
                hbs = []
                for t in range(2):
                    for c in range(NHC):
                        nc.tensor.matmul(
                            ps[:, 512 * t:512 * (t + 1)], lhsT=wsl(t, c),
                            rhs=xbt[c][:, xlo:xlo + 512],
                            start=(c == 0), stop=(c == NHC - 1))
                for t in range(2):
                    hb = wp.tile([128, 512], bf16,
                                 tag=("qh" if t == 0 else "kh"), bufs=2)
                    nc.vector.tensor_scalar(
                        hb[:, :], ps[:, 512 * t:512 * (t + 1)],
                        0.125 if t == 0 else 1.0, bqk[:, t:t + 1],
                        mybir.AluOpType.mult, mybir.AluOpType.add)
                    hbs.append(hb)
                return hbs

            def rope_half(sg, hb, dst):
                sl = slice(512 * sg, 512 * (sg + 1))
                swp = psB.tile([128, 512], f32, tag="acc")
                nc.tensor.matmul(swp[:, :], lhsT=perm[:, :], rhs=hb[:, :],
                                 start=True, stop=True)
                t1 = wp.tile([128, 512], f32, tag="ropet1")
                nc.vector.tensor_mul(t1[:, :], hb[:, :], cs[:, sl])
                t2 = wp.tile([128, 512], f32, tag="ropet2")
                nc.vector.tensor_mul(
                    t2[:, :], swp[:, :], cs[:, SB + 512 * sg:SB + 512 * (sg + 1)])
                nc.vector.tensor_add(dst[:, sl], t1[:, :], t2[:, :])

            def v_st(sg, j, xbt, xlo):
                st = 4 * sg + j
                ps = psB.tile([128, OSL], f32, tag="acc",
                              padded_shape=[128, 512])
                x0 = xlo + 128 * j
                for c in range(NHC):
                    nc.tensor.matmul(
                        ps[:, :], lhsT=xbt[c][:, x0:x0 + 128],
                        rhs=wsl(2, c),
                        start=(c == 0), stop=(c == NHC - 1))
                for h in range(HPC):
                    nc.vector.tensor_copy(
                        vt[:, 2 * st + h, 0:D],
                        ps[:, 64 * h:64 * (h + 1)])

            def proj_sg(sg, xbt, xlo):
                qh, kh = qk_sg2(sg, xbt, xlo)
                rope_half(sg, qh, qr)
                rope_half(sg, kh, kr)
                for j in range(4):
                    v_st(sg, j, xbt, xlo)

            def attn_pass(b, qs):
                q0 = S * b + 512 * qs
                cpsA = psB.tile([128, 512], f32, tag="acc")
                cpsB = psB.tile([128, 512], f32, tag="acc")
                for ks in range(16):
                    k0 = S * b + 128 * ks
                    kb = 16 * b + ks
                    sps = psA.tile([128, 1024], f32, tag="spsbig")
                    nc.tensor.matmul(
                        sps[:, 0:512], lhsT=kr[0:64, k0:k0 + 128],
                        rhs=qr[0:64, q0:q0 + 512], start=True, stop=True)
                    nc.tensor.matmul(
                        sps[:, 512:1024], lhsT=kr[64:128, k0:k0 + 128],
                        rhs=qr[64:128, q0:q0 + 512], start=True, stop=True)
                    et = ep.tile([128, 1024], bf16, tag="expT", bufs=4)
                    nc.scalar.activation(et[:, :], sps[:, :], AF.Exp)
                    nc.tensor.matmul(
                        cpsA[0:D + 1, :], lhsT=vt[:, 2 * kb, :],
                        rhs=et[:, 0:512], start=(ks == 0), stop=(ks == 15))
                    nc.tensor.matmul(
                        cpsB[0:D + 1, :], lhsT=vt[:, 2 * kb + 1, :],
                        rhs=et[:, 512:1024],
                        start=(ks == 0), stop=(ks == 15))

                for h, cps in ((0, cpsA), (1, cpsB)):
                    rs65 = ep.tile([65, 512], f32, tag="rec65")
                    nc.vector.tensor_copy(rs65[64:65, :], cps[64:65, :])
                    rsP = ep.tile([128, 4], f32, tag="rsP")
                    nc.sync.dma_start(out=rsP[:, :], in_=rs65[64:65, :])
                    rPr = ep.tile([128, 4], f32, tag="rPr")
                    nc.vector.reciprocal(rPr[:, :], rsP[:, :])
                    rec0 = ep.tile([1, 512], f32, tag="rec0")
                    nc.sync.dma_start(out=rec0[:, :], in_=rPr[:, :])
                    rb = ep.tile([64, 512], f32, tag="recb")
                    nc.gpsimd.partition_broadcast(rb[:, :], rec0[:, :])
                    nc.vector.tensor_mul(
                        ctxh[h][:, q0:q0 + 512], cps[0:64, :], rb[:, :])
                    nc.gpsimd.dma_start(
                        out=a2a_in[4 * b + qs, 64 * h:64 * (h + 1), :],
                        in_=ctxh[h][:, q0:q0 + 512])

            # ---------- phase A: batch-0 projections + rope ----------
            for sg in range(4):
                proj_sg(sg, xq[0], 512 * sg)

            # tiny warmup collective: pre-arms ncfw so the real AllToAll's
            # trigger-to-start latency is paid here, off the critical path
            warm_in = nc.dram_tensor("warm_in", [NC, 1, 64], bf16)
            warm_out = nc.dram_tensor("warm_out", [NC, 1, 64], bf16)
            nc.gpsimd.collective_compute(
                "AllToAll", mybir.AluOpType.bypass,
                replica_groups=[list(range(NC))],
                ins=[warm_in.ap().opt()],
                outs=[warm_out.ap().opt()])

            # wo: one fat DMA on the Sync queue during attention.
            wo_sb = pp.tile([128, 8 * HID], bf16, tag="woL", name="wo_sb")
            nc.sync.dma_start(out=wo_sb[:, :], in_=wod[:, :])

            # ---------- phase B: batch-1 projections + rope ----------
            for sg in range(4, 8):
                proj_sg(sg, xq[1], 512 * (sg - 4))

            # ---------- phases C/D: the 8 attention passes ----------
            for b in range(2):
                for qs in range(4):
                    attn_pass(b, qs)

            # ---------- phase E: AllToAll + output projection ----------
            nc.gpsimd.collective_compute(
                "AllToAll", mybir.AluOpType.bypass,
                replica_groups=[list(range(NC))],
                ins=[a2a_in.ap().opt()],
                outs=[a2a_out.ap().opt()])

            _cmB.__exit__(None, None, None)
            _cmA.__exit__(None, None, None)
            _cmO = tc.tile_pool(name="psO", bufs=1, space="PSUM")
            psO = _cmO.__enter__()

            # Keep the PE array warm across the AllToAll wait: a chain of
            # matmuls anchored on the last ctx tile so they cannot run
            # before attention finishes; dead-store keeps it from DCE.
            dumsrc = pp.tile([128, 512], bf16, tag="dumsrc")
            nc.gpsimd.memset(dumsrc[:, :], 0.0)
            nc.vector.tensor_copy(
                dumsrc[0:64, :], ctxh[1][:, SB - 512:SB])
            dum = psO.tile([128, 512], f32, tag="dum", bufs=1)
            for i in range(N_DUMMY):
                nc.tensor.matmul(
                    dum[:, :], lhsT=wo_sb[:, 0:128], rhs=dumsrc[:, :],
                    start=True, stop=True)
            dumr = ep.tile([128, 512], f32, tag="dumr")
            nc.vector.tensor_copy(dumr[:, :], dum[:, :])
            dead = nc.dram_tensor("dead", [128, 512], f32)
            nc.sync.dma_start(out=dead[:, :], in_=dumr[:, :])
            cxs = []
            for c in range(NHC):
                cx = pp.tile([128, RB], bf16, tag=f"cxb{c}", name=f"cxb{c}")
                eng = nc.sync if c % 2 == 0 else nc.gpsimd
                eng.dma_start(out=cx[:, :], in_=a2a_out[c, :, :])
                cxs.append(cx)
            for ot in range(8):
                ops = psO.tile([128, 512], f32, tag="ops", bufs=4)
                for c in range(NHC):
                    nc.tensor.matmul(
                        ops[:, :],
                        lhsT=wo_sb[:, 1024 * c + 128 * ot:
                                   1024 * c + 128 * (ot + 1)],
                        rhs=cxs[c][:, :],
                        start=(c == 0), stop=(c == NHC - 1))
                osb = ep.tile([128, RB], bf16, tag="osb", bufs=3)
                nc.scalar.activation(
                    osb[:, :], ops[:, :], AF.Identity,
                    bias=bo_sb[:, ot:ot + 1], scale=1.0)
                eng = nc.sync if ot % 2 == 0 else nc.gpsimd
                eng.dma_start(
                    out=out_ext[128 * ot:128 * (ot + 1), :], in_=osb[:, :])
            _cmO.__exit__(None, None, None)

    nc.finalize()
    return nc


def _host_tables():
    inv = 1.0 / (ROPE_BASE ** (np.arange(0, D, 2, dtype=np.float64) / D))
    pos = np.arange(S, dtype=np.float64)
    freqs = np.outer(pos, inv)                      # [S, 32]
    emb = np.concatenate([freqs, freqs], axis=-1)   # [S, 64]
    cosT = np.cos(emb).T.astype(np.float32)         # [64, S]
    sinT = np.sin(emb).T.astype(np.float32)
    sinS = np.concatenate([-sinT[:32], sinT[32:]], axis=0)
    cos2 = np.ascontiguousarray(np.tile(cosT, (2, 2)))   # [128, 2S]
    sin2 = np.ascontiguousarray(np.tile(sinS, (2, 2)))
    return cos2, sin2


def _pack_wqkv(Wq, Wk, Wv, sl, bf):
    out = np.empty((128, 3 * 1024), dtype=np.float32)
    for t, W in enumerate((Wq, Wk, Wv)):
        wt = W[sl, :].T.reshape(8, 128, 128)          # [c, p, j]
        out[:, 1024 * t:1024 * (t + 1)] = (
            wt.transpose(1, 0, 2).reshape(128, 1024))
    return np.ascontiguousarray(out).astype(bf)


def kernel(**inputs):
    import ml_dtypes
    from concourse.bass_utils import run_bass_kernel_spmd

    global _cached, _last_in_maps
    if _cached is None:
        _cached = _build_nc()
    nc = _cached

    bf = ml_dtypes.bfloat16
    hs = np.asarray(inputs["hidden_states"], dtype=np.float32)
    Wq = np.asarray(inputs["Wq"], dtype=np.float32)
    bq = np.asarray(inputs["bq"], dtype=np.float32)
    Wk = np.asarray(inputs["Wk"], dtype=np.float32)
    bk = np.asarray(inputs["bk"], dtype=np.float32)
    Wv = np.asarray(inputs["Wv"], dtype=np.float32)
    bv = np.asarray(inputs["bv"], dtype=np.float32)
    Wo = np.asarray(inputs["Wo"], dtype=np.float32)
    bo = np.asarray(inputs["bo"], dtype=np.float32)

    cos2, sin2 = _host_tables()
    cs = np.ascontiguousarray(
        np.concatenate([cos2, sin2], axis=1)).astype(bf)   # [128, 2SB]
    bo2 = bo + bv @ Wo.T                                 # fold v-bias exactly
    bo2m = np.ascontiguousarray(bo2.reshape(8, 128).T)   # [128, 8]
    xTfull = np.ascontiguousarray(
        np.concatenate([hs[0].T, hs[1].T], axis=1)).astype(bf)  # [1024, 4096]
    woL = np.ascontiguousarray(
        Wo.T.reshape(8, 128, 1024).transpose(1, 0, 2).reshape(128, 8192)
    ).astype(bf)
    pidx = np.arange(128)
    pm = np.where(pidx % 64 < 32, pidx + 32, pidx - 32)
    permM = np.zeros((128, 128), dtype=np.float32)
    permM[pm, pidx] = 1.0                                # [k, m]: k==perm(m)
    permM = permM.astype(bf)

    in_maps = []
    for c in range(NC):
        sl = slice(OSL * c, OSL * (c + 1))
        bqk = np.stack([bq[sl] * 0.125, bk[sl]], axis=1)  # [128, 2]
        in_maps.append({
            "xT": xTfull,
            "wqkv": _pack_wqkv(Wq, Wk, Wv, sl, bf),
            "woL": woL,
            "bqk": np.ascontiguousarray(bqk.astype(np.float32)),
            "bo2": bo2m,
            "cs": cs,
            "perm": permM,
        })

    _last_in_maps = in_maps
    res = run_bass_kernel_spmd(nc, in_maps, core_ids=list(range(NC)))
    out = np.empty((2, S, HID), dtype=np.float32)
    for c in range(NC):
        b, g = divmod(c, 4)
        out[b, RB * g:RB * (g + 1), :] = res.results[c]["out"].T.astype(np.float32)
    return out


# revision 15
# speedup vs baseline: 1.1910x; 1.1910x over previous
"""AttentionWithRoPE distributed Trainium2 kernel (8 NeuronCores).

Sharding: pure 8-way tensor parallel over heads (2 heads = 128 hidden cols
per core), both batches on every core (seq concatenated to 4096 cols).
Everything stays transposed ([feature, seq] layouts) so no on-device
transposes are needed anywhere.

The kernel is ScalarE-bound: exp of the 2x[2048,2048] score matrices is
~142us of ACTIVATE at 1 elem/lane/cycle. The schedule keeps ScalarE
saturated from ~30us on:
  phase A: project q/k/v + rope for batch 0 (seq groups 0-3).
  phase C: 4 attention passes for batch 0. ALL batch-1 projection work
           (q/k matmuls, rope, v) is spread as small insertions BETWEEN
           KEY-STEPS INSIDE the passes (between passes ScalarE has no exp
           backlog, so any block there stalls it 1:1).
  phase D: 4 attention passes for batch 1, with keep-warm dummy matmuls
           in every key-step: without them the ACT-gated PE micro-idles
           every ~700ns, HAM re-throttles the PE clock to 1.2GHz, and the
           slowed matmuls then gate ACT (observed k=4/8 / k=13/16
           oscillation, never 8/8).
  phase E: AllToAll + keep-warm matmul chain + output projection.

Every dma_start costs ~600ns of SWDGE descriptor time ON THE ISSUING
ENGINE, so DMAs are few and fat, spread over the Sync/GpSimd/ScalarE
queues: x streams as 16x [128,2048] chunks over all three, wq|wk|wv are
host-packed into one [128,3072] load, Wo into one [128,8192], cos|sin in
4 pieces, biases in one. The rope half-rotation (a 32-row partition swap,
which DVE cannot do: ops must keep operand start-partitions equal) is a
PE matmul against a host-supplied 0/1 permutation matrix; the sin-multiply
reads the swapped copy straight from PSUM.

Attention details:
  - scores^T = kT.T @ qT per (head, batch) in [ks, qs] layout as K=64
    row-tiled matmul pairs: head0 streams through PE rows 0-63 while head1
    streams through rows 64-127 concurrently (auto tile_position (0,0) /
    (64,0) from the operands' base partitions; verified dStart ~3ns).
  - exp on ScalarE in [128,1024] ops over 2-bank psum score tiles (~1113ns
    each; 1536-wide ops measure WORSE - 3-bank PSUM reads pay ~230ns).
  - ctx^T via M=65 matmuls with a ones-column appended to V (the 65th
    column gives the softmax denominator for free). V lives in a single 3D
    tile [128, 64, 65] (slot = key-block*2+head); ones columns initialized
    by ONE strided memset.
  - normalization: rowsum (psum partition 64) -> sbuf, DMA-reshape to
    [128,4] so reciprocal runs 128 lanes wide (single-partition reciprocal
    is ~8 cyc/elem!), DMA back, GpSimd partition-broadcast, one DVE
    multiply (fuses psum->sbuf copy + cast). Hop DMAs ride the idle Sync
    queue.
  - PSUM: 2x 2-bank score slots + 3x 1-bank accumulator slots (q/k proj,
    rope swap, v psums, the two ctx accumulators) + 1 dummy bank = 8.
  - AllToAll (bf16, all 8 cores) exchanges 512-row blocks of ctx^T;
    received slabs are exactly the o-chunks the output projection consumes.
  - output projection with full Wo produces out^T [1024, 512] for this
    core's 512 global rows; host transposes back (free).
Bias folds (host side): v-bias folds into the output bias exactly (softmax
rows sum to 1); q is pre-scaled by 1/sqrt(64) inside its bias-copy.
Compute dtype bf16 (fp32 PSUM accumulation).
"""

import numpy as np

HID = 1024
S = 2048
SB = 2 * S       # both batches, seq-concatenated
NHEAD = 16
D = 64
HPC = 2          # heads per core
OSL = 128        # hidden slice per core (HPC * D)
RB = 512         # global row block per core after AllToAll
NC = 8
ROPE_BASE = 10000.0

_cached = None
_last_in_maps = None

N_DUMMY = 150    # keep-PE-warm matmuls spanning the AllToAll wait


def _build_nc():
    import concourse.bacc as bacc
    import concourse.mybir as mybir
    from concourse import tile

    f32 = mybir.dt.float32
    bf16 = mybir.dt.bfloat16
    AF = mybir.ActivationFunctionType

    nc = bacc.Bacc(None, target_bir_lowering=False)

    xT = nc.declare_dram_parameter("xT", [HID, SB], bf16, isOutput=False)
    wqkvd = nc.declare_dram_parameter("wqkv", [128, 3 * HID], bf16,
                                      isOutput=False)
    wod = nc.declare_dram_parameter("woL", [128, 8 * HID], bf16,
                                    isOutput=False)
    bqkd = nc.declare_dram_parameter("bqk", [128, 2], f32, isOutput=False)
    bod = nc.declare_dram_parameter("bo2", [128, 8], f32, isOutput=False)
    csd = nc.declare_dram_parameter("cs", [128, 2 * SB], bf16,
                                    isOutput=False)
    permd = nc.declare_dram_parameter("perm", [128, 128], bf16,
                                      isOutput=False)
    out_ext = nc.declare_dram_parameter("out", [HID, RB], bf16, isOutput=True)

    a2a_in = nc.dram_tensor("a2a_in", [NC, OSL, RB], bf16)
    a2a_out = nc.dram_tensor("a2a_out", [NC, OSL, RB], bf16)

    NHC = HID // 128  # 8 hidden chunks
    QENG = None       # set inside

    with tile.TileContext(nc) as tc:
        with (
            tc.tile_pool(name="persist", bufs=1) as pp,
            tc.tile_pool(name="xs", bufs=16) as xp,
            tc.tile_pool(name="work", bufs=2) as wp,
            tc.tile_pool(name="exp", bufs=2) as ep,
        ):
            # ---------- consts: fat DMAs spread over the 3 queues ---------
            wqkv = pp.tile([128, 3 * HID], bf16, tag="wqkv", name="wqkv")
            nc.scalar.dma_start(out=wqkv[:, :], in_=wqkvd[:, :])
            bqk = pp.tile([128, 2], f32, tag="bqk", name="bqk")
            nc.scalar.dma_start(out=bqk[:, :], in_=bqkd[:, :])
            perm = pp.tile([128, 128], bf16, tag="perm", name="perm")
            nc.scalar.dma_start(out=perm[:, :], in_=permd[:, :])
            cs = pp.tile([128, 2 * SB], bf16, tag="cs", name="cs")
            for half in range(2):          # cos-b0, sin-b0, cos-b1, sin-b1
                for part in range(2):
                    lo = SB * part + S * half
                    nc.scalar.dma_start(out=cs[:, lo:lo + S],
                                        in_=csd[:, lo:lo + S])
            bo_sb = pp.tile([128, 8], f32, tag="bo", name="bo")
            nc.scalar.dma_start(out=bo_sb[:, :], in_=bod[:, :])

            def wsl(t, c):      # lhsT slice for projection t in (q,k,v)
                return wqkv[:, 1024 * t + 128 * c:1024 * t + 128 * (c + 1)]

            # x: 16 fat [128,2048] chunks over 3 queues; pair 0 = batch 0
            # columns, pair 1 = batch 1.
            xq = {}
            for p in range(2):
                tiles = []
                for c in range(NHC):
                    xb = xp.tile([128, 2048], bf16, tag="xb", bufs=16)
                    (nc.sync if c < 4 else nc.gpsimd).dma_start(
                        out=xb[:, :],
                        in_=xT[128 * c:128 * (c + 1),
                               2048 * p:2048 * (p + 1)])
                    tiles.append(xb)
                xq[p] = tiles

            # PSUM pools (8 banks exactly):
            #  psA "spsbig": 2x [128,1024] (scores)          -> 4 banks
            #  psB "acc":    3x [128,512]  (proj/swap/ctx)   -> 3 banks
            #  psD "dumA":   1x [128,512]  (keep-warm)       -> 1 bank
            _cmA = tc.tile_pool(name="psA", bufs=2, space="PSUM")
            _cmB = tc.tile_pool(name="psB", bufs=4, space="PSUM")
            psA = _cmA.__enter__()
            psB = _cmB.__enter__()

            qr = pp.tile([128, SB], bf16, tag="qr", name="qr")
            kr = pp.tile([128, SB], bf16, tag="kr", name="kr")
            vt = pp.tile([128, 2 * SB // 128, D + 1], bf16, tag="vt",
                         name="vt")
            nc.gpsimd.memset(vt[:, :, D:D + 1], 1.0)
            ctxh = [pp.tile([64, SB], bf16, tag=f"ctx{h}", name=f"ctx{h}")
                    for h in range(HPC)]

            # ---------- emission helpers (all emit small blocks) ----------
            # qh/kh: post-bias pre-rope 512-col blocks, consumed by rope.
            def qk_sg2(sg, xbt, xlo):
                ps = psA.tile([128, 1024], f32, tag="spsbig",
                              name=f"qkps{sg}")
                hbs = []
                for t in range(2):
                    for c in range(NHC):
                        nc.tensor.matmul(
                            ps[:, 512 * t:512 * (t + 1)], lhsT=wsl(t, c),
                            rhs=xbt[c][:, xlo:xlo + 512],
                            start=(c == 0), stop=(c == NHC - 1))
                for t in range(2):
                    hb = wp.tile([128, 512], bf16,
                                 tag=("qh" if t == 0 else "kh"), bufs=2)
                    nc.vector.tensor_scalar(
                        hb[:, :], ps[:, 512 * t:512 * (t + 1)],
                        0.125 if t == 0 else 1.0, bqk[:, t:t + 1],
                        mybir.AluOpType.mult, mybir.AluOpType.add)
                    hbs.append(hb)
                return hbs

            def rope_half(sg, hb, dst):
                sl = slice(512 * sg, 512 * (sg + 1))
                swp = psB.tile([128, 512], f32, tag="acc")
                nc.tensor.matmul(swp[:, :], lhsT=perm[:, :], rhs=hb[:, :],
                                 start=True, stop=True)
                t1 = wp.tile([128, 512], f32, tag="ropet1")
                nc.vector.tensor_mul(t1[:, :], hb[:, :], cs[:, sl])
                t2 = wp.tile([128, 512], f32, tag="ropet2")
                nc.vector.tensor_mul(
                    t2[:, :], swp[:, :], cs[:, SB + 512 * sg:SB + 512 * (sg + 1)])
                nc.vector.tensor_add(dst[:, sl], t1[:, :], t2[:, :])

            def v_st(sg, j, xbt, xlo):
                st = 4 * sg + j
                ps = psB.tile([128, OSL], f32, tag="acc",
                              padded_shape=[128, 512])
                x0 = xlo + 128 * j
                for c in range(NHC):
                    nc.tensor.matmul(
                        ps[:, :], lhsT=xbt[c][:, x0:x0 + 128],
                        rhs=wsl(2, c),
                        start=(c == 0), stop=(c == NHC - 1))
                for h in range(HPC):
                    nc.vector.tensor_copy(
                        vt[:, 2 * st + h, 0:D],
                        ps[:, 64 * h:64 * (h + 1)])

            def proj_sg(sg, xbt, xlo):
                qh, kh = qk_sg2(sg, xbt, xlo)
                rope_half(sg, qh, qr)
                rope_half(sg, kh, kr)
                for j in range(4):
                    v_st(sg, j, xbt, xlo)

            def attn_pass(b, qs):
                q0 = S * b + 512 * qs
                cpsA = psB.tile([128, 512], f32, tag="acc")
                cpsB = psB.tile([128, 512], f32, tag="acc")
                for ks in range(16):
                    k0 = S * b + 128 * ks
                    kb = 16 * b + ks
                    sps = psA.tile([128, 1024], f32, tag="spsbig")
                    nc.tensor.matmul(
                        sps[:, 0:512], lhsT=kr[0:64, k0:k0 + 128],
                        rhs=qr[0:64, q0:q0 + 512], start=True, stop=True)
                    nc.tensor.matmul(
                        sps[:, 512:1024], lhsT=kr[64:128, k0:k0 + 128],
                        rhs=qr[64:128, q0:q0 + 512], start=True, stop=True)
                    et = ep.tile([128, 1024], bf16, tag="expT", bufs=4)
                    nc.scalar.activation(et[:, :], sps[:, :], AF.Exp)
                    nc.tensor.matmul(
                        cpsA[0:D + 1, :], lhsT=vt[:, 2 * kb, :],
                        rhs=et[:, 0:512], start=(ks == 0), stop=(ks == 15))
                    nc.tensor.matmul(
                        cpsB[0:D + 1, :], lhsT=vt[:, 2 * kb + 1, :],
                        rhs=et[:, 512:1024],
                        start=(ks == 0), stop=(ks == 15))

                for h, cps in ((0, cpsA), (1, cpsB)):
                    rs65 = ep.tile([65, 512], f32, tag="rec65")
                    nc.vector.tensor_copy(rs65[64:65, :], cps[64:65, :])
                    rsP = ep.tile([128, 4], f32, tag="rsP")
                    nc.sync.dma_start(out=rsP[:, :], in_=rs65[64:65, :])
                    rPr = ep.tile([128, 4], f32, tag="rPr")
                    nc.vector.reciprocal(rPr[:, :], rsP[:, :])
                    rec0 = ep.tile([1, 512], f32, tag="rec0")
                    nc.sync.dma_start(out=rec0[:, :], in_=rPr[:, :])
                    rb = ep.tile([64, 512], f32, tag="recb")
                    nc.gpsimd.partition_broadcast(rb[:, :], rec0[:, :])
                    nc.vector.tensor_mul(
                        ctxh[h][:, q0:q0 + 512], cps[0:64, :], rb[:, :])
                    nc.gpsimd.dma_start(
                        out=a2a_in[4 * b + qs, 64 * h:64 * (h + 1), :],
                        in_=ctxh[h][:, q0:q0 + 512])

            # ---------- phase A: batch-0 projections + rope ----------
            for sg in range(4):
                proj_sg(sg, xq[0], 512 * sg)

            # tiny warmup collective: pre-arms ncfw so the real AllToAll's
            # trigger-to-start latency is paid here, off the critical path
            warm_in = nc.dram_tensor("warm_in", [NC, 1, 64], bf16)
            warm_out = nc.dram_tensor("warm_out", [NC, 1, 64], bf16)
            nc.gpsimd.collective_compute(
                "AllToAll", mybir.AluOpType.bypass,
                replica_groups=[list(range(NC))],
                ins=[warm_in.ap().opt()],
                outs=[warm_out.ap().opt()])

            # wo: one fat DMA on the Sync queue during attention.
            wo_sb = pp.tile([128, 8 * HID], bf16, tag="woL", name="wo_sb")
            nc.sync.dma_start(out=wo_sb[:, :], in_=wod[:, :])

            # ---------- phase B: batch-1 projections + rope ----------
            for sg in range(4, 8):
                proj_sg(sg, xq[1], 512 * (sg - 4))

            # ---------- phases C/D: the 8 attention passes ----------
            for b in range(2):
                for qs in range(4):
                    attn_pass(b, qs)

            # ---------- phase E: AllToAll + output projection ----------
            nc.gpsimd.collective_compute(
                "AllToAll", mybir.AluOpType.bypass,
                replica_groups=[list(range(NC))],
                ins=[a2a_in.ap().opt()],
                outs=[a2a_out.ap().opt()])

            _cmB.__exit__(None, None, None)
            _cmA.__exit__(None, None, None)
            _cmO = tc.tile_pool(name="psO", bufs=1, space="PSUM")
            psO = _cmO.__enter__()

            # Keep the PE array warm across the AllToAll wait: a chain of
            # matmuls anchored on the last ctx tile so they cannot run
            # before attention finishes; dead-store keeps it from DCE.
            dumsrc = pp.tile([128, 512], bf16, tag="dumsrc")
            nc.gpsimd.memset(dumsrc[:, :], 0.0)
            nc.vector.tensor_copy(
                dumsrc[0:64, :], ctxh[1][:, SB - 512:SB])
            dum = psO.tile([128, 512], f32, tag="dum", bufs=1)
            for i in range(N_DUMMY):
                nc.tensor.matmul(
                    dum[:, :], lhsT=wo_sb[:, 0:128], rhs=dumsrc[:, :],
                    start=True, stop=True)
            dumr = ep.tile([128, 512], f32, tag="dumr")
            nc.vector.tensor_copy(dumr[:, :], dum[:, :])
            dead = nc.dram_tensor("dead", [128, 512], f32)
            nc.sync.dma_start(out=dead[:, :], in_=dumr[:, :])
            cxs = []
            for c in range(NHC):
                cx = pp.tile([128, RB], bf16, tag=f"cxb{c}", name=f"cxb{c}")
                eng = nc.sync if c % 2 == 0 else nc.gpsimd
                eng.dma_start(out=cx[:, :], in_=a2a_out[c, :, :])
                cxs.append(cx)
            for ot in range(8):
                ops = psO.tile([128, 512], f32, tag="ops", bufs=4)
                for c in range(NHC):
                    nc.tensor.matmul(
                        ops[:, :],
                        lhsT=wo_sb[:, 1024 * c + 128 * ot:
                                   1024 * c + 128 * (ot + 1)],
                        rhs=cxs[c][:, :],
                        start=(c == 0), stop=(c == NHC - 1))
                osb = ep.tile([128, RB], bf16, tag="osb", bufs=3)
                nc.scalar.activation(
                    osb[:, :], ops[:, :], AF.Identity,
                    bias=bo_sb[:, ot:ot + 1], scale=1.0)
                eng = nc.sync if ot % 2 == 0 else nc.gpsimd
                eng.dma_start(
                    out=out_ext[128 * ot:128 * (ot + 1), :], in_=osb[:, :])
            _cmO.__exit__(None, None, None)

    nc.finalize()
    return nc


def _host_tables():
    inv = 1.0 / (ROPE_BASE ** (np.arange(0, D, 2, dtype=np.float64) / D))
    pos = np.arange(S, dtype=np.float64)
    freqs = np.outer(pos, inv)                      # [S, 32]
    emb = np.concatenate([freqs, freqs], axis=-1)   # [S, 64]
    cosT = np.cos(emb).T.astype(np.float32)         # [64, S]
    sinT = np.sin(emb).T.astype(np.float32)
    sinS = np.concatenate([-sinT[:32], sinT[32:]], axis=0)
    cos2 = np.ascontiguousarray(np.tile(cosT, (2, 2)))   # [128, 2S]
    sin2 = np.ascontiguousarray(np.tile(sinS, (2, 2)))
    return cos2, sin2


def _pack_wqkv(Wq, Wk, Wv, sl, bf):
    out = np.empty((128, 3 * 1024), dtype=np.float32)
    for t, W in enumerate((Wq, Wk, Wv)):
        wt = W[sl, :].T.reshape(8, 128, 128)          # [c, p, j]
        out[:, 1024 * t:1024 * (t + 1)] = (
            wt.transpose(1, 0, 2).reshape(128, 1024))
    return np.ascontiguousarray(out).astype(bf)


def kernel(**inputs):
    import ml_dtypes
    from concourse.bass_utils import run_bass_kernel_spmd

    global _cached, _last_in_maps
    if _cached is None:
        _cached = _build_nc()
    nc = _cached

    bf = ml_dtypes.bfloat16
    hs = np.asarray(inputs["hidden_states"], dtype=np.float32)
    Wq = np.asarray(inputs["Wq"], dtype=np.float32)
    bq = np.asarray(inputs["bq"], dtype=np.float32)
    Wk = np.asarray(inputs["Wk"], dtype=np.float32)
    bk = np.asarray(inputs["bk"], dtype=np.float32)
    Wv = np.asarray(inputs["Wv"], dtype=np.float32)
    bv = np.asarray(inputs["bv"], dtype=np.float32)
    Wo = np.asarray(inputs["Wo"], dtype=np.float32)
    bo = np.asarray(inputs["bo"], dtype=np.float32)

    cos2, sin2 = _host_tables()
    cs = np.ascontiguousarray(
        np.concatenate([cos2, sin2], axis=1)).astype(bf)   # [128, 2SB]
    bo2 = bo + bv @ Wo.T                                 # fold v-bias exactly
    bo2m = np.ascontiguousarray(bo2.reshape(8, 128).T)   # [128, 8]
    xTfull = np.ascontiguousarray(
        np.concatenate([hs[0].T, hs[1].T], axis=1)).astype(bf)  # [1024, 4096]
    woL = np.ascontiguousarray(
        Wo.T.reshape(8, 128, 1024).transpose(1, 0, 2).reshape(128, 8192)
    ).astype(bf)
    pidx = np.arange(128)
    pm = np.where(pidx % 64 < 32, pidx + 32, pidx - 32)
    permM = np.zeros((128, 128), dtype=np.float32)
    permM[pm, pidx] = 1.0                                # [k, m]: k==perm(m)
    permM = permM.astype(bf)

    in_maps = []
    for c in range(NC):
        sl = slice(OSL * c, OSL * (c + 1))
        bqk = np.stack([bq[sl] * 0.125, bk[sl]], axis=1)  # [128, 2]
        in_maps.append({
            "xT": xTfull,
            "wqkv": _pack_wqkv(Wq, Wk, Wv, sl, bf),
            "woL": woL,
            "bqk": np.ascontiguousarray(bqk.astype(np.float32)),
            "bo2": bo2m,
            "cs": cs,
            "perm": permM,
        })

    _last_in_maps = in_maps
    res = run_bass_kernel_spmd(nc, in_maps, core_ids=list(range(NC)))
    out = np.empty((2, S, HID), dtype=np.float32)
    for c in range(NC):
        b, g = divmod(c, 4)
        out[b, RB * g:RB * (g + 1), :] = res.results[c]["out"].T.astype(np.float32)
    return out


# revision 17
# speedup vs baseline: 1.3197x; 1.1080x over previous
"""AttentionWithRoPE distributed Trainium2 kernel (8 NeuronCores).

Sharding: pure 8-way tensor parallel over heads (2 heads = 128 hidden cols
per core), both batches on every core (seq concatenated to 4096 cols).
Everything stays transposed ([feature, seq] layouts) so no on-device
transposes are needed anywhere.

The kernel is ScalarE-bound: exp of the 2x[2048,2048] score matrices is
~142us of ACTIVATE at 1 elem/lane/cycle. The schedule keeps ScalarE
saturated from ~30us on:
  phase A: project q/k/v + rope for batch 0 (seq groups 0-3).
  phase C: 4 attention passes for batch 0. ALL batch-1 projection work
           (q/k matmuls, rope, v) is spread as small insertions BETWEEN
           KEY-STEPS INSIDE the passes (between passes ScalarE has no exp
           backlog, so any block there stalls it 1:1).
  phase D: 4 attention passes for batch 1, with keep-warm dummy matmuls
           in every key-step: without them the ACT-gated PE micro-idles
           every ~700ns, HAM re-throttles the PE clock to 1.2GHz, and the
           slowed matmuls then gate ACT (observed k=4/8 / k=13/16
           oscillation, never 8/8).
  phase E: AllToAll + keep-warm matmul chain + output projection.

Every dma_start costs ~600ns of SWDGE descriptor time ON THE ISSUING
ENGINE, so DMAs are few and fat, spread over the Sync/GpSimd/ScalarE
queues: x streams as 16x [128,2048] chunks over all three, wq|wk|wv are
host-packed into one [128,3072] load, Wo into one [128,8192], cos|sin in
4 pieces, biases in one. The rope half-rotation (a 32-row partition swap,
which DVE cannot do: ops must keep operand start-partitions equal) is a
PE matmul against a host-supplied 0/1 permutation matrix; the sin-multiply
reads the swapped copy straight from PSUM.

Attention details:
  - scores^T = kT.T @ qT per (head, batch) in [ks, qs] layout as K=64
    row-tiled matmul pairs: head0 streams through PE rows 0-63 while head1
    streams through rows 64-127 concurrently (auto tile_position (0,0) /
    (64,0) from the operands' base partitions; verified dStart ~3ns).
  - exp on ScalarE in [128,1024] ops over 2-bank psum score tiles (~1113ns
    each; 1536-wide ops measure WORSE - 3-bank PSUM reads pay ~230ns).
  - ctx^T via M=65 matmuls with a ones-column appended to V (the 65th
    column gives the softmax denominator for free). V lives in a single 3D
    tile [128, 64, 65] (slot = key-block*2+head); ones columns initialized
    by ONE strided memset.
  - normalization: rowsum (psum partition 64) -> sbuf, DMA-reshape to
    [128,4] so reciprocal runs 128 lanes wide (single-partition reciprocal
    is ~8 cyc/elem!), DMA back, GpSimd partition-broadcast, one DVE
    multiply (fuses psum->sbuf copy + cast). Hop DMAs ride the idle Sync
    queue.
  - PSUM: 2x 2-bank score slots + 3x 1-bank accumulator slots (q/k proj,
    rope swap, v psums, the two ctx accumulators) + 1 dummy bank = 8.
  - AllToAll (bf16, all 8 cores) exchanges 512-row blocks of ctx^T;
    received slabs are exactly the o-chunks the output projection consumes.
  - output projection with full Wo produces out^T [1024, 512] for this
    core's 512 global rows; host transposes back (free).
Bias folds (host side): v-bias folds into the output bias exactly (softmax
rows sum to 1); q is pre-scaled by 1/sqrt(64) inside its bias-copy.
Compute dtype bf16 (fp32 PSUM accumulation).
"""

import numpy as np

HID = 1024
S = 2048
SB = 2 * S       # both batches, seq-concatenated
NHEAD = 16
D = 64
HPC = 2          # heads per core
OSL = 128        # hidden slice per core (HPC * D)
RB = 512         # global row block per core after AllToAll
NC = 8
ROPE_BASE = 10000.0

_cached = None
_last_in_maps = None

N_DUMMY = 70    # keep-PE-warm matmuls spanning the AllToAll wait


def _build_nc():
    import concourse.bacc as bacc
    import concourse.mybir as mybir
    from concourse import tile

    f32 = mybir.dt.float32
    bf16 = mybir.dt.bfloat16
    AF = mybir.ActivationFunctionType

    nc = bacc.Bacc(None, target_bir_lowering=False)

    xT = nc.declare_dram_parameter("xT", [HID, SB], bf16, isOutput=False)
    wqkvd = nc.declare_dram_parameter("wqkv", [128, 3 * HID], bf16,
                                      isOutput=False)
    wod = nc.declare_dram_parameter("woL", [128, 8 * HID], bf16,
                                    isOutput=False)
    bqkd = nc.declare_dram_parameter("bqk", [128, 2], f32, isOutput=False)
    bod = nc.declare_dram_parameter("bo2", [128, 8], f32, isOutput=False)
    csd = nc.declare_dram_parameter("cs", [128, 2 * SB], bf16,
                                    isOutput=False)
    permd = nc.declare_dram_parameter("perm", [128, 128], bf16,
                                      isOutput=False)
    out_ext = nc.declare_dram_parameter("out", [HID, RB], bf16, isOutput=True)

    a2a_in = nc.dram_tensor("a2a_in", [NC, OSL, RB], bf16)
    a2a_out = nc.dram_tensor("a2a_out", [NC, OSL, RB], bf16)

    NHC = HID // 128  # 8 hidden chunks
    QENG = None       # set inside

    with tile.TileContext(nc) as tc:
        with (
            tc.tile_pool(name="persist", bufs=1) as pp,
            tc.tile_pool(name="xs", bufs=16) as xp,
            tc.tile_pool(name="work", bufs=2) as wp,
            tc.tile_pool(name="exp", bufs=2) as ep,
        ):
            # ---------- consts: fat DMAs spread over the 3 queues ---------
            wqkv = pp.tile([128, 3 * HID], bf16, tag="wqkv", name="wqkv")
            nc.scalar.dma_start(out=wqkv[:, :], in_=wqkvd[:, :])
            bqk = pp.tile([128, 2], f32, tag="bqk", name="bqk")
            nc.scalar.dma_start(out=bqk[:, :], in_=bqkd[:, :])
            perm = pp.tile([128, 128], bf16, tag="perm", name="perm")
            nc.scalar.dma_start(out=perm[:, :], in_=permd[:, :])
            cs = pp.tile([128, 2 * SB], bf16, tag="cs", name="cs")
            for half in range(2):          # cos-b0, sin-b0, cos-b1, sin-b1
                for part in range(2):
                    lo = SB * part + S * half
                    nc.scalar.dma_start(out=cs[:, lo:lo + S],
                                        in_=csd[:, lo:lo + S])
            bo_sb = pp.tile([128, 8], f32, tag="bo", name="bo")
            nc.scalar.dma_start(out=bo_sb[:, :], in_=bod[:, :])

            def wsl(t, c):      # lhsT slice for projection t in (q,k,v)
                return wqkv[:, 1024 * t + 128 * c:1024 * t + 128 * (c + 1)]

            # x: 16 fat [128,2048] chunks over 3 queues; pair 0 = batch 0
            # columns, pair 1 = batch 1.
            xq = {}
            for p in range(2):
                tiles = []
                for c in range(NHC):
                    xb = xp.tile([128, 2048], bf16, tag="xb", bufs=16)
                    (nc.sync if c < 4 else nc.gpsimd).dma_start(
                        out=xb[:, :],
                        in_=xT[128 * c:128 * (c + 1),
                               2048 * p:2048 * (p + 1)])
                    tiles.append(xb)
                xq[p] = tiles

            # PSUM pools (8 banks exactly):
            #  psA "spsbig": 2x [128,1024] (scores)          -> 4 banks
            #  psB "acc":    3x [128,512]  (proj/swap/ctx)   -> 3 banks
            #  psD "dumA":   1x [128,512]  (keep-warm)       -> 1 bank
            _cmA = tc.tile_pool(name="psA", bufs=2, space="PSUM")
            _cmB = tc.tile_pool(name="psB", bufs=4, space="PSUM")
            psA = _cmA.__enter__()
            psB = _cmB.__enter__()

            qr = pp.tile([128, SB], bf16, tag="qr", name="qr")
            kr = pp.tile([128, SB], bf16, tag="kr", name="kr")
            vt = pp.tile([128, 2 * SB // 128, D + 1], bf16, tag="vt",
                         name="vt")
            nc.gpsimd.memset(vt[:, :, D:D + 1], 1.0)
            ctxh = [pp.tile([64, SB], bf16, tag=f"ctx{h}", name=f"ctx{h}")
                    for h in range(HPC)]

            # ---------- emission helpers (all emit small blocks) ----------
            # qh/kh: post-bias pre-rope 512-col blocks, consumed by rope.
            def rope_half(sg, hb, dst):
                sl = slice(512 * sg, 512 * (sg + 1))
                swp = psB.tile([128, 512], f32, tag="acc")
                nc.tensor.matmul(swp[:, :], lhsT=perm[:, :], rhs=hb[:, :],
                                 start=True, stop=True)
                t1 = wp.tile([128, 512], f32, tag="ropet1")
                nc.vector.tensor_mul(t1[:, :], hb[:, :], cs[:, sl])
                t2 = wp.tile([128, 512], f32, tag="ropet2")
                nc.vector.tensor_mul(
                    t2[:, :], swp[:, :],
                    cs[:, SB + 512 * sg:SB + 512 * (sg + 1)])
                nc.vector.tensor_add(dst[:, sl], t1[:, :], t2[:, :])

            def granules_for(sg, xbt, xlo):
                """Projection work for one seq group as a list of ~2-matmul
                closures, small enough to slip into the PE's per-key-step
                slack under the exp-bound attention passes."""
                st8 = {}
                gl = []

                def qk_gr(t, g):
                    def run():
                        if g == 0:
                            st8[t] = psB.tile([128, 512], f32, tag="acc",
                                              name=f"qkps{sg}_{t}")
                        ps = st8[t]
                        for c in (2 * g, 2 * g + 1):
                            nc.tensor.matmul(
                                ps[:, :], lhsT=wsl(t, c),
                                rhs=xbt[c][:, xlo:xlo + 512],
                                start=(c == 0), stop=(c == NHC - 1))
                        if g == 3:
                            hb = wp.tile([128, 512], bf16,
                                         tag=("qh" if t == 0 else "kh"),
                                         bufs=2)
                            nc.vector.tensor_scalar(
                                hb[:, :], ps[:, :],
                                0.125 if t == 0 else 1.0, bqk[:, t:t + 1],
                                mybir.AluOpType.mult, mybir.AluOpType.add)
                            st8[f"hb{t}"] = hb
                    return run

                def rope_gr(t):
                    def run():
                        rope_half(sg, st8[f"hb{t}"], qr if t == 0 else kr)
                    return run

                def v_gr(j, g):
                    def run():
                        if g == 0:
                            st8[f"v{j}"] = psB.tile(
                                [128, OSL], f32, tag="acc",
                                padded_shape=[128, 512],
                                name=f"vps{sg}_{j}")
                        ps = st8[f"v{j}"]
                        x0 = xlo + 128 * j
                        for c in (2 * g, 2 * g + 1):
                            nc.tensor.matmul(
                                ps[:, :], lhsT=xbt[c][:, x0:x0 + 128],
                                rhs=wsl(2, c),
                                start=(c == 0), stop=(c == NHC - 1))
                        if g == 3:
                            st = 4 * sg + j
                            for h in range(HPC):
                                nc.vector.tensor_copy(
                                    vt[:, 2 * st + h, 0:D],
                                    ps[:, 64 * h:64 * (h + 1)])
                    return run

                for t in (0, 1):
                    for g in range(4):
                        gl.append(qk_gr(t, g))
                    gl.append(rope_gr(t))
                for j in range(4):
                    for g in range(4):
                        gl.append(v_gr(j, g))
                return gl

            def proj_sg(sg, xbt, xlo):
                for f in granules_for(sg, xbt, xlo):
                    f()

            def attn_pass(b, qs, granules=()):
                gq = list(granules)
                q0 = S * b + 512 * qs
                cpsA = psB.tile([128, 512], f32, tag="acc")
                cpsB = psB.tile([128, 512], f32, tag="acc")
                for ks in range(16):
                    k0 = S * b + 128 * ks
                    kb = 16 * b + ks
                    sps = psA.tile([128, 1024], f32, tag="spsbig")
                    nc.tensor.matmul(
                        sps[:, 0:512], lhsT=kr[0:64, k0:k0 + 128],
                        rhs=qr[0:64, q0:q0 + 512], start=True, stop=True)
                    nc.tensor.matmul(
                        sps[:, 512:1024], lhsT=kr[64:128, k0:k0 + 128],
                        rhs=qr[64:128, q0:q0 + 512], start=True, stop=True)
                    et = ep.tile([128, 1024], bf16, tag="expT", bufs=4)
                    nc.scalar.activation(et[:, :], sps[:, :], AF.Exp)
                    nc.tensor.matmul(
                        cpsA[0:D + 1, :], lhsT=vt[:, 2 * kb, :],
                        rhs=et[:, 0:512], start=(ks == 0), stop=(ks == 15))
                    nc.tensor.matmul(
                        cpsB[0:D + 1, :], lhsT=vt[:, 2 * kb + 1, :],
                        rhs=et[:, 512:1024],
                        start=(ks == 0), stop=(ks == 15))
                    for _ in range(2):
                        if gq:
                            gq.pop(0)()
                while gq:
                    gq.pop(0)()
                for h, cps in ((0, cpsA), (1, cpsB)):
                    rs65 = ep.tile([65, 512], f32, tag="rec65")
                    nc.vector.tensor_copy(rs65[64:65, :], cps[64:65, :])
                    rsP = ep.tile([128, 4], f32, tag="rsP")
                    nc.sync.dma_start(out=rsP[:, :], in_=rs65[64:65, :])
                    rPr = ep.tile([128, 4], f32, tag="rPr")
                    nc.vector.reciprocal(rPr[:, :], rsP[:, :])
                    rec0 = ep.tile([1, 512], f32, tag="rec0")
                    nc.sync.dma_start(out=rec0[:, :], in_=rPr[:, :])
                    rb = ep.tile([64, 512], f32, tag="recb")
                    nc.gpsimd.partition_broadcast(rb[:, :], rec0[:, :])
                    nc.vector.tensor_mul(
                        ctxh[h][:, q0:q0 + 512], cps[0:64, :], rb[:, :])
                    nc.gpsimd.dma_start(
                        out=a2a_in[4 * b + qs, 64 * h:64 * (h + 1), :],
                        in_=ctxh[h][:, q0:q0 + 512])

            # ---------- phase A: batch-0 projections + rope ----------
            for sg in range(4):
                proj_sg(sg, xq[0], 512 * sg)

            # tiny warmup collective: pre-arms ncfw so the real AllToAll's
            # trigger-to-start latency is paid here, off the critical path
            warm_in = nc.dram_tensor("warm_in", [NC, 1, 64], bf16)
            warm_out = nc.dram_tensor("warm_out", [NC, 1, 64], bf16)
            nc.gpsimd.collective_compute(
                "AllToAll", mybir.AluOpType.bypass,
                replica_groups=[list(range(NC))],
                ins=[warm_in.ap().opt()],
                outs=[warm_out.ap().opt()])

            # wo: one fat DMA on the Sync queue during attention.
            wo_sb = pp.tile([128, 8 * HID], bf16, tag="woL", name="wo_sb")
            nc.sync.dma_start(out=wo_sb[:, :], in_=wod[:, :])

            # ---------- phases C/D: attention; batch-1 projections are
            # drained as micro-granules inside the batch-0 passes ----------
            for i in range(4):
                attn_pass(0, i, granules_for(4 + i, xq[1], 512 * i))
            for qs in range(4):
                attn_pass(1, qs)

            # ---------- phase E: AllToAll + output projection ----------
            nc.gpsimd.collective_compute(
                "AllToAll", mybir.AluOpType.bypass,
                replica_groups=[list(range(NC))],
                ins=[a2a_in.ap().opt()],
                outs=[a2a_out.ap().opt()])

            _cmB.__exit__(None, None, None)
            _cmA.__exit__(None, None, None)
            _cmO = tc.tile_pool(name="psO", bufs=1, space="PSUM")
            psO = _cmO.__enter__()

            # Keep the PE array warm across the AllToAll wait: a chain of
            # matmuls anchored on the last ctx tile so they cannot run
            # before attention finishes; dead-store keeps it from DCE.
            dumsrc = pp.tile([128, 512], bf16, tag="dumsrc")
            nc.gpsimd.memset(dumsrc[:, :], 0.0)
            nc.vector.tensor_copy(
                dumsrc[0:64, :], ctxh[1][:, SB - 512:SB])
            dum = psO.tile([128, 512], f32, tag="dum", bufs=1)
            for i in range(N_DUMMY):
                nc.tensor.matmul(
                    dum[:, :], lhsT=wo_sb[:, 0:128], rhs=dumsrc[:, :],
                    start=True, stop=True)
            dumr = ep.tile([128, 512], f32, tag="dumr")
            nc.vector.tensor_copy(dumr[:, :], dum[:, :])
            dead = nc.dram_tensor("dead", [128, 512], f32)
            nc.sync.dma_start(out=dead[:, :], in_=dumr[:, :])
            cxs = []
            for c in range(NHC):
                cx = pp.tile([128, RB], bf16, tag=f"cxb{c}", name=f"cxb{c}")
                eng = nc.sync if c % 2 == 0 else nc.gpsimd
                eng.dma_start(out=cx[:, :], in_=a2a_out[c, :, :])
                cxs.append(cx)
            for ot in range(8):
                ops = psO.tile([128, 512], f32, tag="ops", bufs=4)
                for c in range(NHC):
                    nc.tensor.matmul(
                        ops[:, :],
                        lhsT=wo_sb[:, 1024 * c + 128 * ot:
                                   1024 * c + 128 * (ot + 1)],
                        rhs=cxs[c][:, :],
                        start=(c == 0), stop=(c == NHC - 1))
                osb = ep.tile([128, RB], bf16, tag="osb", bufs=3)
                nc.scalar.activation(
                    osb[:, :], ops[:, :], AF.Identity,
                    bias=bo_sb[:, ot:ot + 1], scale=1.0)
                eng = nc.sync if ot % 2 == 0 else nc.gpsimd
                eng.dma_start(
                    out=out_ext[128 * ot:128 * (ot + 1), :], in_=osb[:, :])
            _cmO.__exit__(None, None, None)

    nc.finalize()
    return nc


def _host_tables():
    inv = 1.0 / (ROPE_BASE ** (np.arange(0, D, 2, dtype=np.float64) / D))
    pos = np.arange(S, dtype=np.float64)
    freqs = np.outer(pos, inv)                      # [S, 32]
    emb = np.concatenate([freqs, freqs], axis=-1)   # [S, 64]
    cosT = np.cos(emb).T.astype(np.float32)         # [64, S]
    sinT = np.sin(emb).T.astype(np.float32)
    sinS = np.concatenate([-sinT[:32], sinT[32:]], axis=0)
    cos2 = np.ascontiguousarray(np.tile(cosT, (2, 2)))   # [128, 2S]
    sin2 = np.ascontiguousarray(np.tile(sinS, (2, 2)))
    return cos2, sin2


def _pack_wqkv(Wq, Wk, Wv, sl, bf):
    out = np.empty((128, 3 * 1024), dtype=np.float32)
    for t, W in enumerate((Wq, Wk, Wv)):
        wt = W[sl, :].T.reshape(8, 128, 128)          # [c, p, j]
        out[:, 1024 * t:1024 * (t + 1)] = (
            wt.transpose(1, 0, 2).reshape(128, 1024))
    return np.ascontiguousarray(out).astype(bf)


def kernel(**inputs):
    import ml_dtypes
    from concourse.bass_utils import run_bass_kernel_spmd

    global _cached, _last_in_maps
    if _cached is None:
        _cached = _build_nc()
    nc = _cached

    bf = ml_dtypes.bfloat16
    hs = np.asarray(inputs["hidden_states"], dtype=np.float32)
    Wq = np.asarray(inputs["Wq"], dtype=np.float32)
    bq = np.asarray(inputs["bq"], dtype=np.float32)
    Wk = np.asarray(inputs["Wk"], dtype=np.float32)
    bk = np.asarray(inputs["bk"], dtype=np.float32)
    Wv = np.asarray(inputs["Wv"], dtype=np.float32)
    bv = np.asarray(inputs["bv"], dtype=np.float32)
    Wo = np.asarray(inputs["Wo"], dtype=np.float32)
    bo = np.asarray(inputs["bo"], dtype=np.float32)

    cos2, sin2 = _host_tables()
    cs = np.ascontiguousarray(
        np.concatenate([cos2, sin2], axis=1)).astype(bf)   # [128, 2SB]
    bo2 = bo + bv @ Wo.T                                 # fold v-bias exactly
    bo2m = np.ascontiguousarray(bo2.reshape(8, 128).T)   # [128, 8]
    xTfull = np.ascontiguousarray(
        np.concatenate([hs[0].T, hs[1].T], axis=1)).astype(bf)  # [1024, 4096]
    woL = np.ascontiguousarray(
        Wo.T.reshape(8, 128, 1024).transpose(1, 0, 2).reshape(128, 8192)
    ).astype(bf)
    pidx = np.arange(128)
    pm = np.where(pidx % 64 < 32, pidx + 32, pidx - 32)
    permM = np.zeros((128, 128), dtype=np.float32)
    permM[pm, pidx] = 1.0                                # [k, m]: k==perm(m)
    permM = permM.astype(bf)

    in_maps = []
    for c in range(NC):
        sl = slice(OSL * c, OSL * (c + 1))
        bqk = np.stack([bq[sl] * 0.125, bk[sl]], axis=1)  # [128, 2]
        in_maps.append({
            "xT": xTfull,
            "wqkv": _pack_wqkv(Wq, Wk, Wv, sl, bf),
            "woL": woL,
            "bqk": np.ascontiguousarray(bqk.astype(np.float32)),
            "bo2": bo2m,
            "cs": cs,
            "perm": permM,
        })

    _last_in_maps = in_maps
    res = run_bass_kernel_spmd(nc, in_maps, core_ids=list(range(NC)))
    out = np.empty((2, S, HID), dtype=np.float32)
    for c in range(NC):
        b, g = divmod(c, 4)
        out[b, RB * g:RB * (g + 1), :] = res.results[c]["out"].T.astype(np.float32)
    return out


# revision 18
# speedup vs baseline: 1.3490x; 1.0223x over previous
"""AttentionWithRoPE distributed Trainium2 kernel (8 NeuronCores).

Sharding: pure 8-way tensor parallel over heads (2 heads = 128 hidden cols
per core), both batches on every core (seq concatenated to 4096 cols).
Everything stays transposed ([feature, seq] layouts) so no on-device
transposes are needed anywhere.

The kernel is ScalarE-bound: exp of the 2x[2048,2048] score matrices is
~142us of ACTIVATE at 1 elem/lane/cycle. The schedule keeps ScalarE
saturated from ~30us on:
  phase A: project q/k/v + rope for batch 0 (seq groups 0-3).
  phase C: 4 attention passes for batch 0. ALL batch-1 projection work
           (q/k matmuls, rope, v) is spread as small insertions BETWEEN
           KEY-STEPS INSIDE the passes (between passes ScalarE has no exp
           backlog, so any block there stalls it 1:1).
  phase D: 4 attention passes for batch 1, with keep-warm dummy matmuls
           in every key-step: without them the ACT-gated PE micro-idles
           every ~700ns, HAM re-throttles the PE clock to 1.2GHz, and the
           slowed matmuls then gate ACT (observed k=4/8 / k=13/16
           oscillation, never 8/8).
  phase E: AllToAll + keep-warm matmul chain + output projection.

Every dma_start costs ~600ns of SWDGE descriptor time ON THE ISSUING
ENGINE, so DMAs are few and fat, spread over the Sync/GpSimd/ScalarE
queues: x streams as 16x [128,2048] chunks over all three, wq|wk|wv are
host-packed into one [128,3072] load, Wo into one [128,8192], cos|sin in
4 pieces, biases in one. The rope half-rotation (a 32-row partition swap,
which DVE cannot do: ops must keep operand start-partitions equal) is a
PE matmul against a host-supplied 0/1 permutation matrix; the sin-multiply
reads the swapped copy straight from PSUM.

Attention details:
  - scores^T = kT.T @ qT per (head, batch) in [ks, qs] layout as K=64
    row-tiled matmul pairs: head0 streams through PE rows 0-63 while head1
    streams through rows 64-127 concurrently (auto tile_position (0,0) /
    (64,0) from the operands' base partitions; verified dStart ~3ns).
  - exp on ScalarE in [128,1024] ops over 2-bank psum score tiles (~1113ns
    each; 1536-wide ops measure WORSE - 3-bank PSUM reads pay ~230ns).
  - ctx^T via M=65 matmuls with a ones-column appended to V (the 65th
    column gives the softmax denominator for free). V lives in a single 3D
    tile [128, 64, 65] (slot = key-block*2+head); ones columns initialized
    by ONE strided memset.
  - normalization: rowsum (psum partition 64) -> sbuf, DMA-reshape to
    [128,4] so reciprocal runs 128 lanes wide (single-partition reciprocal
    is ~8 cyc/elem!), DMA back, GpSimd partition-broadcast, one DVE
    multiply (fuses psum->sbuf copy + cast). Hop DMAs ride the idle Sync
    queue.
  - PSUM: 2x 2-bank score slots + 3x 1-bank accumulator slots (q/k proj,
    rope swap, v psums, the two ctx accumulators) + 1 dummy bank = 8.
  - AllToAll (bf16, all 8 cores) exchanges 512-row blocks of ctx^T;
    received slabs are exactly the o-chunks the output projection consumes.
  - output projection with full Wo produces out^T [1024, 512] for this
    core's 512 global rows; host transposes back (free).
Bias folds (host side): v-bias folds into the output bias exactly (softmax
rows sum to 1); q is pre-scaled by 1/sqrt(64) inside its bias-copy.
Compute dtype bf16 (fp32 PSUM accumulation).
"""

import numpy as np

HID = 1024
S = 2048
SB = 2 * S       # both batches, seq-concatenated
NHEAD = 16
D = 64
HPC = 2          # heads per core
OSL = 128        # hidden slice per core (HPC * D)
RB = 512         # global row block per core after AllToAll
NC = 8
ROPE_BASE = 10000.0

_cached = None
_last_in_maps = None

N_DUMMY = 50    # keep-PE-warm matmuls spanning the AllToAll wait


def _build_nc():
    import concourse.bacc as bacc
    import concourse.mybir as mybir
    from concourse import tile

    f32 = mybir.dt.float32
    bf16 = mybir.dt.bfloat16
    AF = mybir.ActivationFunctionType

    nc = bacc.Bacc(None, target_bir_lowering=False)

    xT = nc.declare_dram_parameter("xT", [HID, SB], bf16, isOutput=False)
    wqkvd = nc.declare_dram_parameter("wqkv", [128, 3 * HID], bf16,
                                      isOutput=False)
    wod = nc.declare_dram_parameter("woL", [128, 8 * HID], bf16,
                                    isOutput=False)
    bqkd = nc.declare_dram_parameter("bqk", [128, 2], f32, isOutput=False)
    bod = nc.declare_dram_parameter("bo2", [128, 8], f32, isOutput=False)
    csd = nc.declare_dram_parameter("cs", [128, 2 * SB], bf16,
                                    isOutput=False)
    permd = nc.declare_dram_parameter("perm", [128, 128], bf16,
                                      isOutput=False)
    out_ext = nc.declare_dram_parameter("out", [HID, RB], bf16, isOutput=True)

    a2a_in = nc.dram_tensor("a2a_in", [NC, OSL, RB], bf16)
    a2a_out = nc.dram_tensor("a2a_out", [NC, OSL, RB], bf16)

    NHC = HID // 128  # 8 hidden chunks
    QENG = None       # set inside

    with tile.TileContext(nc) as tc:
        with (
            tc.tile_pool(name="persist", bufs=1) as pp,
            tc.tile_pool(name="xs", bufs=16) as xp,
            tc.tile_pool(name="work", bufs=2) as wp,
            tc.tile_pool(name="exp", bufs=2) as ep,
        ):
            # ---------- consts: fat DMAs spread over the 3 queues ---------
            wqkv = pp.tile([128, 3 * HID], bf16, tag="wqkv", name="wqkv")
            nc.scalar.dma_start(out=wqkv[:, :], in_=wqkvd[:, :])
            bqk = pp.tile([128, 2], f32, tag="bqk", name="bqk")
            nc.scalar.dma_start(out=bqk[:, :], in_=bqkd[:, :])
            perm = pp.tile([128, 128], bf16, tag="perm", name="perm")
            nc.scalar.dma_start(out=perm[:, :], in_=permd[:, :])
            cs = pp.tile([128, 2 * SB], bf16, tag="cs", name="cs")
            for half in range(2):          # cos-b0, sin-b0, cos-b1, sin-b1
                for part in range(2):
                    lo = SB * part + S * half
                    nc.scalar.dma_start(out=cs[:, lo:lo + S],
                                        in_=csd[:, lo:lo + S])
            bo_sb = pp.tile([128, 8], f32, tag="bo", name="bo")
            nc.scalar.dma_start(out=bo_sb[:, :], in_=bod[:, :])

            def wsl(t, c):      # lhsT slice for projection t in (q,k,v)
                return wqkv[:, 1024 * t + 128 * c:1024 * t + 128 * (c + 1)]

            # x: 16 fat [128,2048] chunks over 3 queues; pair 0 = batch 0
            # columns, pair 1 = batch 1.
            xq = {}
            for p in range(2):
                tiles = []
                for c in range(NHC):
                    xb = xp.tile([128, 2048], bf16, tag="xb", bufs=16)
                    (nc.sync if c < 4 else nc.gpsimd).dma_start(
                        out=xb[:, :],
                        in_=xT[128 * c:128 * (c + 1),
                               2048 * p:2048 * (p + 1)])
                    tiles.append(xb)
                xq[p] = tiles

            # PSUM pools (8 banks exactly):
            #  psA "spsbig": 2x [128,1024] (scores)          -> 4 banks
            #  psB "acc":    3x [128,512]  (proj/swap/ctx)   -> 3 banks
            #  psD "dumA":   1x [128,512]  (keep-warm)       -> 1 bank
            _cmA = tc.tile_pool(name="psA", bufs=2, space="PSUM")
            _cmB = tc.tile_pool(name="psB", bufs=4, space="PSUM")
            psA = _cmA.__enter__()
            psB = _cmB.__enter__()

            qr = pp.tile([128, SB], bf16, tag="qr", name="qr")
            kr = pp.tile([128, SB], bf16, tag="kr", name="kr")
            vt = pp.tile([128, 2 * SB // 128, D + 1], bf16, tag="vt",
                         name="vt")
            nc.gpsimd.memset(vt[:, :, D:D + 1], 1.0)
            ctxh = [pp.tile([64, SB], bf16, tag=f"ctx{h}", name=f"ctx{h}")
                    for h in range(HPC)]

            # ---------- emission helpers (all emit small blocks) ----------
            # qh/kh: post-bias pre-rope 512-col blocks, consumed by rope.
            def rope_half(sg, hb, dst):
                sl = slice(512 * sg, 512 * (sg + 1))
                swp = psB.tile([128, 512], f32, tag="acc")
                nc.tensor.matmul(swp[:, :], lhsT=perm[:, :], rhs=hb[:, :],
                                 start=True, stop=True)
                t1 = wp.tile([128, 512], f32, tag="ropet1")
                nc.vector.tensor_mul(t1[:, :], hb[:, :], cs[:, sl])
                t2 = wp.tile([128, 512], f32, tag="ropet2")
                nc.vector.tensor_mul(
                    t2[:, :], swp[:, :],
                    cs[:, SB + 512 * sg:SB + 512 * (sg + 1)])
                nc.vector.tensor_add(dst[:, sl], t1[:, :], t2[:, :])

            def granules_for(sg, xbt, xlo):
                """Projection work for one seq group as a list of ~2-matmul
                closures, small enough to slip into the PE's per-key-step
                slack under the exp-bound attention passes."""
                st8 = {}
                gl = []

                def qk_gr(t, g):
                    def run():
                        if g == 0:
                            st8[t] = psB.tile([128, 512], f32, tag="acc",
                                              name=f"qkps{sg}_{t}")
                        ps = st8[t]
                        for c in (2 * g, 2 * g + 1):
                            nc.tensor.matmul(
                                ps[:, :], lhsT=wsl(t, c),
                                rhs=xbt[c][:, xlo:xlo + 512],
                                start=(c == 0), stop=(c == NHC - 1))
                        if g == 3:
                            hb = wp.tile([128, 512], bf16,
                                         tag=("qh" if t == 0 else "kh"),
                                         bufs=2)
                            nc.vector.tensor_scalar(
                                hb[:, :], ps[:, :],
                                0.125 if t == 0 else 1.0, bqk[:, t:t + 1],
                                mybir.AluOpType.mult, mybir.AluOpType.add)
                            st8[f"hb{t}"] = hb
                    return run

                def rope_gr(t):
                    def run():
                        rope_half(sg, st8[f"hb{t}"], qr if t == 0 else kr)
                    return run

                def v_gr(j, g):
                    def run():
                        if g == 0:
                            st8[f"v{j}"] = psB.tile(
                                [128, OSL], f32, tag="acc",
                                padded_shape=[128, 512],
                                name=f"vps{sg}_{j}")
                        ps = st8[f"v{j}"]
                        x0 = xlo + 128 * j
                        for c in (2 * g, 2 * g + 1):
                            nc.tensor.matmul(
                                ps[:, :], lhsT=xbt[c][:, x0:x0 + 128],
                                rhs=wsl(2, c),
                                start=(c == 0), stop=(c == NHC - 1))
                        if g == 3:
                            st = 4 * sg + j
                            for h in range(HPC):
                                nc.vector.tensor_copy(
                                    vt[:, 2 * st + h, 0:D],
                                    ps[:, 64 * h:64 * (h + 1)])
                    return run

                for t in (0, 1):
                    for g in range(4):
                        gl.append(qk_gr(t, g))
                    gl.append(rope_gr(t))
                for j in range(4):
                    for g in range(4):
                        gl.append(v_gr(j, g))
                return gl

            def proj_sg(sg, xbt, xlo):
                for f in granules_for(sg, xbt, xlo):
                    f()

            def attn_pass(b, qs, granules=()):
                gq = list(granules)
                q0 = S * b + 512 * qs
                cpsA = psB.tile([128, 512], f32, tag="acc")
                cpsB = psB.tile([128, 512], f32, tag="acc")
                for ks in range(16):
                    k0 = S * b + 128 * ks
                    kb = 16 * b + ks
                    sps = psA.tile([128, 1024], f32, tag="spsbig")
                    nc.tensor.matmul(
                        sps[:, 0:512], lhsT=kr[0:64, k0:k0 + 128],
                        rhs=qr[0:64, q0:q0 + 512], start=True, stop=True)
                    nc.tensor.matmul(
                        sps[:, 512:1024], lhsT=kr[64:128, k0:k0 + 128],
                        rhs=qr[64:128, q0:q0 + 512], start=True, stop=True)
                    et = ep.tile([128, 1024], bf16, tag="expT", bufs=4)
                    nc.scalar.activation(et[:, :], sps[:, :], AF.Exp)
                    nc.tensor.matmul(
                        cpsA[0:D + 1, :], lhsT=vt[:, 2 * kb, :],
                        rhs=et[:, 0:512], start=(ks == 0), stop=(ks == 15))
                    nc.tensor.matmul(
                        cpsB[0:D + 1, :], lhsT=vt[:, 2 * kb + 1, :],
                        rhs=et[:, 512:1024],
                        start=(ks == 0), stop=(ks == 15))
                    for _ in range(2):
                        if gq:
                            gq.pop(0)()
                while gq:
                    gq.pop(0)()
                for h, cps in ((0, cpsA), (1, cpsB)):
                    # One [65,512] copy to SBUF releases the ctx psum slot
                    # immediately (the next pass's accumulators reuse it
                    # without waiting out the normalization chain); row 64
                    # is the softmax rowsum. Reciprocal via DMA-reshape to
                    # [128,4] so it runs 128 lanes wide (single-partition
                    # reciprocal is ~8 cyc/elem). Hop DMAs ride the idle
                    # Sync queue.
                    cs65 = ep.tile([65, 512], f32, tag="rec65", bufs=3)
                    nc.vector.tensor_copy(cs65[:, :], cps[0:D + 1, :])
                    rsP = ep.tile([128, 4], f32, tag="rsP")
                    nc.sync.dma_start(out=rsP[:, :], in_=cs65[64:65, :])
                    rPr = ep.tile([128, 4], f32, tag="rPr")
                    nc.vector.reciprocal(rPr[:, :], rsP[:, :])
                    rec0 = ep.tile([1, 512], f32, tag="rec0")
                    nc.sync.dma_start(out=rec0[:, :], in_=rPr[:, :])
                    rb = ep.tile([64, 512], f32, tag="recb")
                    nc.gpsimd.partition_broadcast(rb[:, :], rec0[:, :])
                    nc.vector.tensor_mul(
                        ctxh[h][:, q0:q0 + 512], cs65[0:64, :], rb[:, :])
                    nc.gpsimd.dma_start(
                        out=a2a_in[4 * b + qs, 64 * h:64 * (h + 1), :],
                        in_=ctxh[h][:, q0:q0 + 512])

            # ---------- phase A: batch-0 projections + rope ----------
            for sg in range(4):
                proj_sg(sg, xq[0], 512 * sg)

            # tiny warmup collective: pre-arms ncfw so the real AllToAll's
            # trigger-to-start latency is paid here, off the critical path
            warm_in = nc.dram_tensor("warm_in", [NC, 1, 64], bf16)
            warm_out = nc.dram_tensor("warm_out", [NC, 1, 64], bf16)
            nc.gpsimd.collective_compute(
                "AllToAll", mybir.AluOpType.bypass,
                replica_groups=[list(range(NC))],
                ins=[warm_in.ap().opt()],
                outs=[warm_out.ap().opt()])

            # wo: one fat DMA on the Sync queue during attention.
            wo_sb = pp.tile([128, 8 * HID], bf16, tag="woL", name="wo_sb")
            nc.sync.dma_start(out=wo_sb[:, :], in_=wod[:, :])

            # ---------- phases C/D: attention; batch-1 projections are
            # drained as micro-granules inside the batch-0 passes ----------
            for i in range(4):
                attn_pass(0, i, granules_for(4 + i, xq[1], 512 * i))
            for qs in range(4):
                attn_pass(1, qs)

            # ---------- phase E: AllToAll + output projection ----------
            nc.gpsimd.collective_compute(
                "AllToAll", mybir.AluOpType.bypass,
                replica_groups=[list(range(NC))],
                ins=[a2a_in.ap().opt()],
                outs=[a2a_out.ap().opt()])

            _cmB.__exit__(None, None, None)
            _cmA.__exit__(None, None, None)
            _cmO = tc.tile_pool(name="psO", bufs=1, space="PSUM")
            psO = _cmO.__enter__()

            # Keep the PE array warm across the AllToAll wait: a chain of
            # matmuls anchored on the last ctx tile so they cannot run
            # before attention finishes; dead-store keeps it from DCE.
            dumsrc = pp.tile([128, 512], bf16, tag="dumsrc")
            nc.gpsimd.memset(dumsrc[:, :], 0.0)
            nc.vector.tensor_copy(
                dumsrc[0:64, :], ctxh[1][:, SB - 512:SB])
            dum = psO.tile([128, 512], f32, tag="dum", bufs=1)
            for i in range(N_DUMMY):
                nc.tensor.matmul(
                    dum[:, :], lhsT=wo_sb[:, 0:128], rhs=dumsrc[:, :],
                    start=True, stop=True)
            dumr = ep.tile([128, 512], f32, tag="dumr")
            nc.vector.tensor_copy(dumr[:, :], dum[:, :])
            dead = nc.dram_tensor("dead", [128, 512], f32)
            nc.sync.dma_start(out=dead[:, :], in_=dumr[:, :])
            cxs = []
            for c in range(NHC):
                cx = pp.tile([128, RB], bf16, tag=f"cxb{c}", name=f"cxb{c}")
                eng = nc.sync if c % 2 == 0 else nc.gpsimd
                eng.dma_start(out=cx[:, :], in_=a2a_out[c, :, :])
                cxs.append(cx)
            for ot in range(8):
                ops = psO.tile([128, 512], f32, tag="ops", bufs=4)
                for c in range(NHC):
                    nc.tensor.matmul(
                        ops[:, :],
                        lhsT=wo_sb[:, 1024 * c + 128 * ot:
                                   1024 * c + 128 * (ot + 1)],
                        rhs=cxs[c][:, :],
                        start=(c == 0), stop=(c == NHC - 1))
                osb = ep.tile([128, RB], bf16, tag="osb", bufs=3)
                nc.scalar.activation(
                    osb[:, :], ops[:, :], AF.Identity,
                    bias=bo_sb[:, ot:ot + 1], scale=1.0)
                eng = nc.sync if ot % 2 == 0 else nc.gpsimd
                eng.dma_start(
                    out=out_ext[128 * ot:128 * (ot + 1), :], in_=osb[:, :])
            _cmO.__exit__(None, None, None)

    nc.finalize()
    return nc


def _host_tables():
    inv = 1.0 / (ROPE_BASE ** (np.arange(0, D, 2, dtype=np.float64) / D))
    pos = np.arange(S, dtype=np.float64)
    freqs = np.outer(pos, inv)                      # [S, 32]
    emb = np.concatenate([freqs, freqs], axis=-1)   # [S, 64]
    cosT = np.cos(emb).T.astype(np.float32)         # [64, S]
    sinT = np.sin(emb).T.astype(np.float32)
    sinS = np.concatenate([-sinT[:32], sinT[32:]], axis=0)
    cos2 = np.ascontiguousarray(np.tile(cosT, (2, 2)))   # [128, 2S]
    sin2 = np.ascontiguousarray(np.tile(sinS, (2, 2)))
    return cos2, sin2


def _pack_wqkv(Wq, Wk, Wv, sl, bf):
    out = np.empty((128, 3 * 1024), dtype=np.float32)
    for t, W in enumerate((Wq, Wk, Wv)):
        wt = W[sl, :].T.reshape(8, 128, 128)          # [c, p, j]
        out[:, 1024 * t:1024 * (t + 1)] = (
            wt.transpose(1, 0, 2).reshape(128, 1024))
    return np.ascontiguousarray(out).astype(bf)


def kernel(**inputs):
    import ml_dtypes
    from concourse.bass_utils import run_bass_kernel_spmd

    global _cached, _last_in_maps
    if _cached is None:
        _cached = _build_nc()
    nc = _cached

    bf = ml_dtypes.bfloat16
    hs = np.asarray(inputs["hidden_states"], dtype=np.float32)
    Wq = np.asarray(inputs["Wq"], dtype=np.float32)
    bq = np.asarray(inputs["bq"], dtype=np.float32)
    Wk = np.asarray(inputs["Wk"], dtype=np.float32)
    bk = np.asarray(inputs["bk"], dtype=np.float32)
    Wv = np.asarray(inputs["Wv"], dtype=np.float32)
    bv = np.asarray(inputs["bv"], dtype=np.float32)
    Wo = np.asarray(inputs["Wo"], dtype=np.float32)
    bo = np.asarray(inputs["bo"], dtype=np.float32)

    cos2, sin2 = _host_tables()
    cs = np.ascontiguousarray(
        np.concatenate([cos2, sin2], axis=1)).astype(bf)   # [128, 2SB]
    bo2 = bo + bv @ Wo.T                                 # fold v-bias exactly
    bo2m = np.ascontiguousarray(bo2.reshape(8, 128).T)   # [128, 8]
    xTfull = np.ascontiguousarray(
        np.concatenate([hs[0].T, hs[1].T], axis=1)).astype(bf)  # [1024, 4096]
    woL = np.ascontiguousarray(
        Wo.T.reshape(8, 128, 1024).transpose(1, 0, 2).reshape(128, 8192)
    ).astype(bf)
    pidx = np.arange(128)
    pm = np.where(pidx % 64 < 32, pidx + 32, pidx - 32)
    permM = np.zeros((128, 128), dtype=np.float32)
    permM[pm, pidx] = 1.0                                # [k, m]: k==perm(m)
    permM = permM.astype(bf)

    in_maps = []
    for c in range(NC):
        sl = slice(OSL * c, OSL * (c + 1))
        bqk = np.stack([bq[sl] * 0.125, bk[sl]], axis=1)  # [128, 2]
        in_maps.append({
            "xT": xTfull,
            "wqkv": _pack_wqkv(Wq, Wk, Wv, sl, bf),
            "woL": woL,
            "bqk": np.ascontiguousarray(bqk.astype(np.float32)),
            "bo2": bo2m,
            "cs": cs,
            "perm": permM,
        })

    _last_in_maps = in_maps
    res = run_bass_kernel_spmd(nc, in_maps, core_ids=list(range(NC)))
    out = np.empty((2, S, HID), dtype=np.float32)
    for c in range(NC):
        b, g = divmod(c, 4)
        out[b, RB * g:RB * (g + 1), :] = res.results[c]["out"].T.astype(np.float32)
    return out
